# revision 3
# baseline (speedup 1.0000x reference)
"""Trainium2 Bass kernel for nn_BinGATConv (2-layer GAT + LN + mean-pool + MLP).

Strategy (8 NeuronCores, SPMD):
  - Nodes dst-sharded: core c owns dst nodes [c*5000, (c+1)*5000); edges are
    1D-partitioned by dst on the host (index work only) and sorted by
    (dst_block, src_half).
  - 4 sequential SPMD launches; the host only reshards/concats between them:
      P0: per-core slice of the L1 gather table  T1[n] = [h1(n)|1|s1(n)]
      P1: L1 message passing (dma_gather by src + PE one-hot matmul scatter
          into PSUM per 128-dst block) + ReLU/LN + W2 projection -> T2 slice
      P2: L2 message passing + ReLU/LN + per-graph partial mean-pool
      P3: combine 8 partial pools + tiny MLP head (replicated)
  - Per-edge attention: custom DVE op builds the masked score matrix
    SC[e,d] = (d==dstloc_e) ? (d1[d]+s_e) : -200, a second custom DVE op
    applies leaky-relu, ACT exponentiates into the weighted one-hot M (bf16),
    and one PE matmul per 128-edge tile does gather-weight-scatter:
    PSUM[d, 0:F+1] += M[e,d]^T @ [h[src_e] | 1].
"""

import re
from contextlib import ExitStack

import ml_dtypes
import numpy as np

import concourse.bass as bass
import concourse.bacc as bacc
import concourse.mybir as mybir
import concourse.tile as tile
import concourse.dve_ops as dvo
from concourse.dve_spec import Spec, Src0, Src1, C0, C1, C2, eq, maxx, select, Idx
from concourse.bass_utils import run_bass_kernel_spmd

F32 = mybir.dt.float32
BF16 = mybir.dt.bfloat16
I16 = mybir.dt.int16
NPBF = ml_dtypes.bfloat16

NCORES = 8
N = 40000
E = 640000
G = 64
SL = N // NCORES          # 5000 nodes per core
SLP = 5120                # padded slice (40*128)
NB = SLP // 128           # 40 dst blocks per core
LOROWS = 4 * SLP          # 20480 rows in each table half
F1 = 128                  # layer-1 feature dim
F2 = 64                   # layer-2 feature dim
ROW1 = 256                # u16 cols per T1 row (512B)
ROW2 = 128                # u16 cols per T2 row (256B)
GRP = 4                   # dst blocks per gather group
NEG = -200.0              # masked score (exp(0.2*-200)=4e-18 ~ 0)
EPS = 1e-5

_OPS = {}
DBG_STAGE = 0
TRACE = False
LAST_EXEC_NS = 0
EXEC_NS = []
_RUN = run_bass_kernel_spmd


def _register_ops():
    if "GAT_MASK_ANT" in dvo._SUB_OPCODE_FOR_NAME:
        _OPS["mask"] = next(o for o in dvo.OPS if o.name == "GAT_MASK_ANT")
        _OPS["lrelu"] = next(o for o in dvo.OPS if o.name == "GAT_LRELU_ANT")
        return

    def mask_ref(in0, in1, s0, s1, imm2):
        a0 = np.asarray(in0, np.float32).reshape(np.asarray(in0).shape[0], -1)
        a1 = np.asarray(in1, np.float32).reshape(np.asarray(in1).shape[0], -1)
        idx = np.arange(a0.shape[-1], dtype=np.float32)[None, :]
        return np.where(idx == np.asarray(s0, np.float32), a0 + np.asarray(s1, np.float32), a1).astype(np.float32)

    def lrelu_ref(in0, in1, s0, s1, imm2):
        a0 = np.asarray(in0, np.float32).reshape(np.asarray(in0).shape[0], -1)
        return np.maximum(a0, a0 * imm2).astype(np.float32)

    specs = [
        ("GAT_MASK_ANT", select(eq(Idx, C0), Src0 + C1, Src1), mask_ref, "mask"),
        ("GAT_LRELU_ANT", maxx(Src0, Src0 * C2), lrelu_ref, "lrelu"),
    ]
    for name, body, ref, key in specs:
        op = dvo.DveOp(name, Spec(body=body, reference=ref), subdim=False, uops_sha={})
        opc = max(dvo._SUB_OPCODE_FOR_NAME.values()) + 1
        assert opc < 0x20, "custom DVE opcode table full"
        dvo.OPS.append(op)
        dvo._SUB_OPCODE_FOR_NAME[name] = opc
        dvo.CUSTOM_DVE_SPECS[name] = op.spec
        for ver in ("v3",):
            try:
                op.compile(ver)
            except ValueError as e:
                m = re.search(ver + r": ([0-9a-f]+)", str(e))
                if not m:
                    raise
                op.uops_sha[ver] = m.group(1)
            op.compile(ver)
        _OPS[key] = op


# --------------------------------------------------------------------------
# Host-side graph partitioning (pure index work)
# --------------------------------------------------------------------------

def _padded_row(n):
    """Global node id -> row in the padded (8*5120) table layout."""
    return (n // SL) * SLP + (n % SL)


def _prep_plan(edge_index):
    src = np.concatenate([edge_index[0].astype(np.int64), np.arange(N, dtype=np.int64)])
    dst = np.concatenate([edge_index[1].astype(np.int64), np.arange(N, dtype=np.int64)])
    prow = _padded_row(src)

    # per (core, local block, half) edge lists
    seg = {}
    for c in range(NCORES):
        m = (dst >= c * SL) & (dst < (c + 1) * SL)
        sp = prow[m]
        dl = dst[m] - c * SL
        order = np.argsort(dl, kind="stable")
        sp = sp[order]
        dl = dl[order]
        blk = dl // 128
        lo = sp < LOROWS
        for b in range(NB):
            mb = blk == b
            for half, mh in (("lo", mb & lo), ("hi", mb & ~lo)):
                rows = sp[mh] - (0 if half == "lo" else LOROWS)
                seg[(c, b, half)] = (rows.astype(np.int64), (dl[mh] - b * 128).astype(np.int64))

    # common tile structure: per (block, half) max tile count across cores
    ntile = {}
    for b in range(NB):
        for half in ("lo", "hi"):
            mx = max(len(seg[(c, b, half)][0]) for c in range(NCORES))
            ntile[(b, half)] = max(1, -(-mx // 128))

    # global tile order: groups of GRP blocks; within a group all lo tiles
    # (block-major) then all hi tiles
    tiles = []          # (block, half, idx_pos_in_stream)
    lo_tile_of = {}
    hi_tile_of = {}
    nlo = nhi = 0
    groups = []
    for g0 in range(0, NB, GRP):
        blocks = list(range(g0, min(g0 + GRP, NB)))
        g = {"blocks": blocks, "lo0": nlo, "hi0": nhi, "tiles": []}
        for b in blocks:
            lo_tile_of[b] = nlo
            for _ in range(ntile[(b, "lo")]):
                g["tiles"].append((b, "lo", nlo))
                tiles.append((b, "lo", nlo))
                nlo += 1
        for b in blocks:
            hi_tile_of[b] = nhi
            for _ in range(ntile[(b, "hi")]):
                g["tiles"].append((b, "hi", nhi))
                tiles.append((b, "hi", nhi))
                nhi += 1
        g["nlo"] = nlo - g["lo0"]
        g["nhi"] = nhi - g["hi0"]
        groups.append(g)

    ntot = len(tiles)

    # first/last tile (PSUM start/stop) per block, in group-processing order
    first = {}
    last = {}
    for ti, (b, half, _) in enumerate(tiles):
        if b not in first:
            first[b] = ti
        last[b] = ti

    # per-core idx / dstloc arrays
    idx_lo = np.zeros((NCORES, 128, nlo * 8), np.int16)
    idx_hi = np.zeros((NCORES, 128, nhi * 8), np.int16)
    dstloc = np.full((NCORES, 128, ntot), 999.0, np.float32)
    for c in range(NCORES):
        for b in range(NB):
            for half, base_of, arr in (("lo", lo_tile_of, idx_lo), ("hi", hi_tile_of, idx_hi)):
                rows, dl = seg[(c, b, half)]
                nt = ntile[(b, half)]
                rpad = np.zeros(nt * 128, np.int64)
                rpad[: len(rows)] = rows
                wrapped = rpad.reshape(nt * 8, 16).T  # idx i -> [i%16, i//16]
                arr[c, :, base_of[b] * 8: base_of[b] * 8 + nt * 8] = np.tile(
                    wrapped.astype(np.int16), (8, 1))
        for ti, (b, half, _) in enumerate(tiles):
            pass
    # dstloc per tile column (tile order = `tiles`)
    for c in range(NCORES):
        for b in range(NB):
            for half, base_of in (("lo", lo_tile_of), ("hi", hi_tile_of)):
                rows, dl = seg[(c, b, half)]
                nt = ntile[(b, half)]
                dpad = np.full(nt * 128, 999.0, np.float32)
                dpad[: len(dl)] = dl.astype(np.float32)
                # which global tile indices hold this (b, half) stream?
                tis = [ti for ti, (bb, hh, _) in enumerate(tiles) if bb == b and hh == half]
                for k, ti in enumerate(tis):
                    dstloc[c, :, ti] = dpad[k * 128: (k + 1) * 128]

    return {
        "groups": groups, "tiles": tiles, "first": first, "last": last,
        "nlo": nlo, "nhi": nhi, "ntot": ntot,
        "idx_lo": idx_lo, "idx_hi": idx_hi, "dstloc": dstloc,
        "lo_tile_of": lo_tile_of, "hi_tile_of": hi_tile_of,
    }


def _prep_pool(batch):
    """Per-core one-hot graph-membership tiles [NB, 128, G] bf16 (0 for pad)."""
    ghot = np.zeros((NCORES, NB, 128, G), NPBF)
    for c in range(NCORES):
        bslice = batch[c * SL: (c + 1) * SL].astype(np.int64)
        oh = np.zeros((SLP, G), np.float32)
        oh[np.arange(SL), bslice] = 1.0
        ghot[c] = oh.reshape(NB, 128, G).astype(NPBF)
    return ghot


# --------------------------------------------------------------------------
# Program builders
# --------------------------------------------------------------------------

def _new_nc():
    return bacc.Bacc("TRN2", target_bir_lowering=False, debug=False,
                     enable_asserts=False, num_devices=NCORES)


def _build_p0():
    """Per-core slice of T1: rows [h1|1|s1] bf16/f32-packed, plus d1 per block."""
    nc = _new_nc()
    x_in = nc.dram_tensor("xsl", [SLP, F1], F32, kind="ExternalInput").ap()
    w1_in = nc.dram_tensor("W1", [F1, F1], F32, kind="ExternalInput").ap()
    a1s_in = nc.dram_tensor("a1s", [F1], F32, kind="ExternalInput").ap()
    a1d_in = nc.dram_tensor("a1d", [F1], F32, kind="ExternalInput").ap()
    id_in = nc.dram_tensor("ident", [128, 128], F32, kind="ExternalInput").ap()
    t1_out = nc.dram_tensor("t1slice", [SLP, ROW1], BF16, kind="ExternalOutput").ap()
    d1_out = nc.dram_tensor("d1own", [SLP], F32, kind="ExternalOutput").ap()

    with tile.TileContext(nc, num_cores=NCORES) as tc, ExitStack() as ctx:
        singles = ctx.enter_context(tc.tile_pool(name="singles", bufs=1))
        sb = ctx.enter_context(tc.tile_pool(name="sb", bufs=3))
        ps = ctx.enter_context(tc.tile_pool(name="ps", bufs=4, space="PSUM"))

        ident = singles.tile([128, 128], F32)
        nc.sync.dma_start(ident, id_in)
        w1sb = singles.tile([128, F1], F32)
        nc.sync.dma_start(w1sb, w1_in)
        a1s_sb = singles.tile([128, 1], F32)
        nc.sync.dma_start(a1s_sb, a1s_in.rearrange("(a b) -> a b", b=1))
        a1d_sb = singles.tile([128, 1], F32)
        nc.sync.dma_start(a1d_sb, a1d_in.rearrange("(a b) -> a b", b=1))

        # W1T_ext [f, 130] bf16 = [W1^T | W1^T a1s | W1^T a1d]
        w1t_ext = singles.tile([128, F1 + 2], BF16)
        p = ps.tile([128, 128], F32, tag="ps")
        nc.tensor.transpose(p, w1sb, ident)
        nc.scalar.activation(w1t_ext[:, 0:F1], p, mybir.ActivationFunctionType.Copy)
        p2 = ps.tile([128, 1], F32, tag="ps")
        nc.tensor.matmul(p2, w1sb, a1s_sb, start=True, stop=True)
        nc.scalar.activation(w1t_ext[:, F1:F1 + 1], p2, mybir.ActivationFunctionType.Copy)
        p3 = ps.tile([128, 1], F32, tag="ps")
        nc.tensor.matmul(p3, w1sb, a1d_sb, start=True, stop=True)
        nc.scalar.activation(w1t_ext[:, F1 + 1:F1 + 2], p3, mybir.ActivationFunctionType.Copy)

        d1stage = singles.tile([128, NB], F32)
        nc.vector.memset(d1stage, 0.0)

        for t in range(NB):
            xt = sb.tile([128, F1], F32, tag="xt")
            nc.sync.dma_start(xt, x_in[t * 128:(t + 1) * 128, :])
            xps = ps.tile([128, 128], F32, tag="ps")
            nc.tensor.transpose(xps, xt, ident)
            xtb = sb.tile([128, 128], BF16, tag="xtb")
            nc.scalar.activation(xtb, xps, mybir.ActivationFunctionType.Copy)
            hps = ps.tile([128, F1 + 2], F32, tag="ps2")
            nc.tensor.matmul(hps, xtb, w1t_ext, start=True, stop=True)

            rowb = sb.tile([128, ROW1], BF16, tag="rowb")
            nc.vector.memset(rowb, 0.0)
            nc.scalar.activation(rowb[:, 0:F1], hps[:, 0:F1], mybir.ActivationFunctionType.Copy)
            nc.vector.memset(rowb[:, F1:F1 + 1], 1.0)
            rowb32 = rowb.bitcast(F32)
            nc.vector.tensor_copy(rowb32[:, 65:66], hps[:, F1:F1 + 1])
            nc.vector.tensor_copy(d1stage[:, t:t + 1], hps[:, F1 + 1:F1 + 2])
            nc.sync.dma_start(d1_out[t * 128:(t + 1) * 128].rearrange("(a b) -> a b", b=1),
                              d1stage[:, t:t + 1])
            nc.sync.dma_start(t1_out[t * 128:(t + 1) * 128, :], rowb)
    nc.finalize()
    return nc


def _build_msg_layer(plan, layer):
    """P1 (layer=1) / P2 (layer=2): gather + attention + scatter + post."""
    F = F1 if layer == 1 else F2
    ROW = ROW1 if layer == 1 else ROW2
    SCOL = 65 if layer == 1 else 33      # f32 col of s in gathered row
    nc = _new_nc()

    tlo_in = nc.dram_tensor("tlo", [LOROWS, ROW], BF16, kind="ExternalInput").ap()
    thi_in = nc.dram_tensor("thi", [LOROWS, ROW], BF16, kind="ExternalInput").ap()
    ilo_in = nc.dram_tensor("idxlo", [128, plan["nlo"] * 8], I16, kind="ExternalInput").ap()
    ihi_in = nc.dram_tensor("idxhi", [128, plan["nhi"] * 8], I16, kind="ExternalInput").ap()
    dl_in = nc.dram_tensor("dstloc", [128, plan["ntot"]], F32, kind="ExternalInput").ap()
    dd_in = nc.dram_tensor("down", [SLP], F32, kind="ExternalInput").ap()
    b_in = nc.dram_tensor("bias", [F], F32, kind="ExternalInput").ap()
    g_in = nc.dram_tensor("gamma", [F], F32, kind="ExternalInput").ap()
    be_in = nc.dram_tensor("beta", [F], F32, kind="ExternalInput").ap()
    id_in = nc.dram_tensor("ident", [128, 128], F32, kind="ExternalInput").ap()
    if layer == 1:
        w2_in = nc.dram_tensor("W2", [F2, F1], F32, kind="ExternalInput").ap()
        a2s_in = nc.dram_tensor("a2s", [F2], F32, kind="ExternalInput").ap()
        a2d_in = nc.dram_tensor("a2d", [F2], F32, kind="ExternalInput").ap()
        t2_out = nc.dram_tensor("t2slice", [SLP, ROW2], BF16, kind="ExternalOutput").ap()
        d2_out = nc.dram_tensor("d2own", [SLP], F32, kind="ExternalOutput").ap()
    else:
        gh_in = nc.dram_tensor("ghot", [NB, 128, G], BF16, kind="ExternalInput").ap()
        pool_out = nc.dram_tensor("pooled", [G, F2 + 1], F32, kind="ExternalOutput").ap()

    groups, tiles = plan["groups"], plan["tiles"]
    first, last = plan["first"], plan["last"]

    with tile.TileContext(nc, num_cores=NCORES) as tc, ExitStack() as ctx:
        singles = ctx.enter_context(tc.tile_pool(name="singles", bufs=1))
        sb = ctx.enter_context(tc.tile_pool(name="sb", bufs=4))
        import os
        gsb = ctx.enter_context(tc.tile_pool(name="gsb", bufs=int(os.environ.get("GBUFS", "2"))))
        msb = ctx.enter_context(tc.tile_pool(name="msb", bufs=4))
        posb = ctx.enter_context(tc.tile_pool(name="posb", bufs=3))
        agg_ps = ctx.enter_context(tc.tile_pool(name="aggps", bufs=5, space="PSUM"))
        pps = ctx.enter_context(tc.tile_pool(name="pps", bufs=1, space="PSUM")) if layer == 2 else None
        aux_ps = ctx.enter_context(tc.tile_pool(name="auxps", bufs=2, space="PSUM"))

        ident = singles.tile([128, 128], F32)
        nc.sync.dma_start(ident, id_in)
        neg = singles.tile([128, 128], F32)
        nc.vector.memset(neg, NEG)
        ones_row = singles.tile([1, 128], F32)
        nc.vector.memset(ones_row, 1.0)
        eps_col = singles.tile([128, 1], F32)
        nc.vector.memset(eps_col, EPS)

        # broadcast constants [128, F] built via K=1 matmul ones^T @ row
        def bcast_row(dram_row_ap, width, nm):
            t = singles.tile([1, width], F32, tag="bcrow", name=f"bcrow_{nm}")
            nc.sync.dma_start(t, dram_row_ap)
            p = aux_ps.tile([128, width], F32, tag="aux", name=f"bcps_{nm}")
            nc.tensor.matmul(p, ones_row, t[0:1, 0:width], start=True, stop=True)
            out = singles.tile([128, width], F32, name=f"bcast_{nm}")
            nc.scalar.activation(out, p, mybir.ActivationFunctionType.Copy)
            return out

        bB = bcast_row(b_in.rearrange("(a b) -> a b", a=1), F, "b")
        gB = bcast_row(g_in.rearrange("(a b) -> a b", a=1), F, "g")
        beB = bcast_row(be_in.rearrange("(a b) -> a b", a=1), F, "be")

        # d-broadcast tiles per block: dB_all[:, b*128:(b+1)*128]
        d_row = singles.tile([1, SLP], F32)
        nc.sync.dma_start(d_row, dd_in.rearrange("(a b) -> a b", a=1))
        dB_all = singles.tile([128, SLP], F32)
        for b0 in range(0, SLP, 512):
            p = aux_ps.tile([128, 512], F32, tag="aux")
            nc.tensor.matmul(p, ones_row, d_row[0:1, b0:b0 + 512], start=True, stop=True)
            nc.scalar.activation(dB_all[:, b0:b0 + 512], p, mybir.ActivationFunctionType.Copy)

        if layer == 1:
            w2sb = singles.tile([64, F1], F32)
            nc.sync.dma_start(w2sb, w2_in)
            a2s_sb = singles.tile([64, 1], F32)
            nc.sync.dma_start(a2s_sb, a2s_in.rearrange("(a b) -> a b", b=1))
            a2d_sb = singles.tile([64, 1], F32)
            nc.sync.dma_start(a2d_sb, a2d_in.rearrange("(a b) -> a b", b=1))
            w2t_ext = singles.tile([128, F2 + 2], BF16)
            p = aux_ps.tile([128, 64], F32, tag="aux")
            nc.tensor.transpose(p, w2sb, ident[0:64, 0:64])
            nc.scalar.activation(w2t_ext[:, 0:F2], p, mybir.ActivationFunctionType.Copy)
            p2 = aux_ps.tile([128, 1], F32, tag="aux")
            nc.tensor.matmul(p2, w2sb, a2s_sb, start=True, stop=True)
            nc.scalar.activation(w2t_ext[:, F2:F2 + 1], p2, mybir.ActivationFunctionType.Copy)
            p3 = aux_ps.tile([128, 1], F32, tag="aux")
            nc.tensor.matmul(p3, w2sb, a2d_sb, start=True, stop=True)
            nc.scalar.activation(w2t_ext[:, F2 + 1:F2 + 2], p3, mybir.ActivationFunctionType.Copy)
            d2stage = singles.tile([128, NB], F32)
            nc.vector.memset(d2stage, 0.0)
        else:
            pool_psum = pps.tile([G, F2 + 1], F32)

        mask_op, lrelu_op = _OPS["mask"], _OPS["lrelu"]

        def postproc(b, agg):
            zc = posb.tile([128, 1], F32, tag="zc")
            nc.vector.tensor_scalar(zc, agg[:, F:F + 1], 1e-30, None, mybir.AluOpType.max)
            rz = posb.tile([128, 1], F32, tag="rz")
            nc.vector.reciprocal_approx_fast(rz, zc)
            if DBG_STAGE == 6:
                return
            u = posb.tile([128, F], F32, tag="u")
            nc.vector.tensor_scalar(u, agg[:, 0:F], rz, None, mybir.AluOpType.mult)
            u2 = posb.tile([128, F], F32, tag="u2")
            nc.vector.tensor_tensor(u2, u, bB, mybir.AluOpType.add)
            r = posb.tile([128, F], F32, tag="r")
            msum = posb.tile([128, 1], F32, tag="msum")
            nc.scalar.activation(r, u2, mybir.ActivationFunctionType.Relu, accum_out=msum)
            if DBG_STAGE == 7:
                return
            mu = posb.tile([128, 1], F32, tag="mu")
            nc.vector.tensor_scalar(mu, msum, 1.0 / F, None, mybir.AluOpType.mult)
            xc = posb.tile([128, F], F32, tag="xc")
            nc.vector.tensor_scalar(xc, r, mu, None, mybir.AluOpType.subtract)
            if DBG_STAGE == 9:
                return
            scr = posb.tile([128, F], F32, tag="scr")
            vsum = posb.tile([128, 1], F32, tag="vsum")
            nc.scalar.activation(scr, xc, mybir.ActivationFunctionType.Square,
                                 accum_out=vsum)
            if DBG_STAGE == 10:
                return
            sd = posb.tile([128, 1], F32, tag="sd")
            nc.scalar.activation(sd, vsum, mybir.ActivationFunctionType.Sqrt,
                                 bias=eps_col, scale=1.0 / F)
            if DBG_STAGE == 11:
                return
            rsd = posb.tile([128, 1], F32, tag="rsd")
            nc.vector.reciprocal(rsd, sd)
            if DBG_STAGE == 8:
                return
            t1 = posb.tile([128, F], F32, tag="t1")
            nc.vector.tensor_scalar(t1, xc, rsd, None, mybir.AluOpType.mult)
            t2 = posb.tile([128, F], F32, tag="t2")
            nc.vector.tensor_tensor(t2, t1, gB, mybir.AluOpType.mult)
            hb = posb.tile([128, F], F32, tag="hb")
            nc.vector.tensor_tensor(hb, t2, beB, mybir.AluOpType.add)
            if DBG_STAGE == 4:
                return

            if layer == 1:
                lnT_ps = aux_ps.tile([128, F], F32, tag="aux")
                nc.tensor.transpose(lnT_ps, hb, ident)
                lnbT = posb.tile([128, F], BF16, tag="lnbT")
                nc.scalar.activation(lnbT, lnT_ps, mybir.ActivationFunctionType.Copy)
                proj = aux_ps.tile([128, F2 + 2], F32, tag="aux")
                nc.tensor.matmul(proj, lnbT, w2t_ext, start=True, stop=True)
                if DBG_STAGE == 5:
                    return
                rowb = posb.tile([128, ROW2], BF16, tag="rowb")
                nc.vector.memset(rowb, 0.0)
                nc.scalar.activation(rowb[:, 0:F2], proj[:, 0:F2], mybir.ActivationFunctionType.Copy)
                nc.vector.memset(rowb[:, F2:F2 + 1], 1.0)
                rowb32 = rowb.bitcast(F32)
                nc.vector.tensor_copy(rowb32[:, 33:34], proj[:, F2:F2 + 1])
                nc.vector.tensor_copy(d2stage[:, b:b + 1], proj[:, F2 + 1:F2 + 2])
                nc.sync.dma_start(d2_out[b * 128:(b + 1) * 128].rearrange("(a b) -> a b", b=1),
                                  d2stage[:, b:b + 1])
                nc.sync.dma_start(t2_out[b * 128:(b + 1) * 128, :], rowb)
            else:
                hf = posb.tile([128, F2 + 1], BF16, tag="hf")
                nc.scalar.activation(hf[:, 0:F2], hb, mybir.ActivationFunctionType.Copy)
                nc.vector.memset(hf[:, F2:F2 + 1], 1.0)
                gh = posb.tile([128, G], BF16, tag="gh")
                nc.sync.dma_start(gh, gh_in[b, :, :])
                nc.tensor.matmul(pool_psum, gh, hf, start=(b == 0), stop=(b == NB - 1))

        agg_of = {}
        for g in groups:
            nlo_g, nhi_g = g["nlo"], g["nhi"]
            glo = gsb.tile([128, max(nlo_g, 1), ROW], BF16, tag="glo")
            ghi = gsb.tile([128, max(nhi_g, 1), ROW], BF16, tag="ghi")
            ilo_sb = sb.tile([128, max(nlo_g, 1) * 8], I16, tag="ilo")
            ihi_sb = sb.tile([128, max(nhi_g, 1) * 8], I16, tag="ihi")
            dl_sb = sb.tile([128, len(g["tiles"])], F32, tag="dl")
            if nlo_g:
                nc.sync.dma_start(ilo_sb[:, 0:nlo_g * 8],
                                  ilo_in[:, g["lo0"] * 8:(g["lo0"] + nlo_g) * 8])
                nc.gpsimd.dma_gather(glo[:, 0:nlo_g, :], tlo_in, ilo_sb[:, 0:nlo_g * 8],
                                     nlo_g * 128, nlo_g * 128, ROW, single_packet=False)
            if nhi_g:
                nc.sync.dma_start(ihi_sb[:, 0:nhi_g * 8],
                                  ihi_in[:, g["hi0"] * 8:(g["hi0"] + nhi_g) * 8])
                nc.gpsimd.dma_gather(ghi[:, 0:nhi_g, :], thi_in, ihi_sb[:, 0:nhi_g * 8],
                                     nhi_g * 128, nhi_g * 128, ROW, single_packet=False)
            t0 = tiles.index(g["tiles"][0]) if g["tiles"] else 0
            nc.sync.dma_start(dl_sb, dl_in[:, t0:t0 + len(g["tiles"])])

            for k, (b, half, spos) in enumerate(g["tiles"]):
                ti = t0 + k
                if half == "lo":
                    gt = glo[:, spos - g["lo0"], :]
                else:
                    gt = ghi[:, spos - g["hi0"], :]
                gt32 = gt.bitcast(F32)
                if DBG_STAGE == 1:
                    continue
                if b not in agg_of:
                    agg_of[b] = agg_ps.tile([128, F + 1], F32, tag="agg", name=f"agg{b}")
                u = msb.tile([128, 128], F32, tag="u")
                nc.vector._custom_dve(mask_op, out=u,
                                      in0=dB_all[:, b * 128:(b + 1) * 128],
                                      in1=neg,
                                      s0=dl_sb[:, k:k + 1],
                                      s1=gt32[:, SCOL:SCOL + 1])
                lr = msb.tile([128, 128], F32, tag="lr")
                nc.vector._custom_dve(lrelu_op, out=lr, in0=u, imm2=0.2)
                m = msb.tile([128, 128], BF16, tag="m")
                nc.scalar.activation(m, lr, mybir.ActivationFunctionType.Exp)
                if DBG_STAGE == 2:
                    agg_of.pop(b)
                    continue
                nc.tensor.matmul(agg_of[b], m, gt[:, 0:F + 1],
                                 start=(ti == first[b]), stop=(ti == last[b]))
                if ti == last[b]:
                    if DBG_STAGE == 3:
                        agg_of.pop(b)
                    else:
                        postproc(b, agg_of.pop(b))

        if layer == 2:
            pout = singles.tile([G, F2 + 1], F32)
            nc.vector.tensor_copy(pout, pool_psum)
            nc.sync.dma_start(pool_out, pout)
    nc.finalize()
    return nc


def _build_p3():
    nc = _new_nc()
    pin = nc.dram_tensor("pall", [G, NCORES * (F2 + 1)], F32, kind="ExternalInput").ap()
    wl_in = nc.dram_tensor("Wl", [F2, F2], F32, kind="ExternalInput").ap()
    bl_in = nc.dram_tensor("bl", [F2], F32, kind="ExternalInput").ap()
    wc_in = nc.dram_tensor("Wc", [1, F2], F32, kind="ExternalInput").ap()
    bc_in = nc.dram_tensor("bc", [1], F32, kind="ExternalInput").ap()
    id_in = nc.dram_tensor("ident", [128, 128], F32, kind="ExternalInput").ap()
    out = nc.dram_tensor("out", [G], F32, kind="ExternalOutput").ap()

    with tile.TileContext(nc, num_cores=NCORES) as tc, ExitStack() as ctx:
        singles = ctx.enter_context(tc.tile_pool(name="singles", bufs=1))
        ps = ctx.enter_context(tc.tile_pool(name="ps", bufs=4, space="PSUM"))

        ident = singles.tile([128, 128], F32)
        nc.sync.dma_start(ident, id_in)
        acc = singles.tile([G, (F2 + 1) * NCORES], F32)
        nc.sync.dma_start(acc, pin)
        tots = [singles.tile([G, F2 + 1], F32, tag=f"tot{i}", name=f"tot{i}") for i in range(NCORES - 1)]
        nc.vector.tensor_tensor(tots[0], acc[:, 0:F2 + 1], acc[:, F2 + 1:2 * (F2 + 1)],
                                mybir.AluOpType.add)
        for c in range(2, NCORES):
            nc.vector.tensor_tensor(tots[c - 1], tots[c - 2],
                                    acc[:, c * (F2 + 1):(c + 1) * (F2 + 1)],
                                    mybir.AluOpType.add)
        tot = tots[NCORES - 2]
        cnt = singles.tile([G, 1], F32)
        nc.vector.tensor_scalar(cnt, tot[:, F2:F2 + 1], 1.0, None, mybir.AluOpType.max)
        rc = singles.tile([G, 1], F32)
        nc.vector.reciprocal(rc, cnt)
        pm = singles.tile([G, F2], F32)
        nc.vector.tensor_scalar(pm, tot[:, 0:F2], rc, None, mybir.AluOpType.mult)
        pmT_ps = ps.tile([F2, G], F32, tag="ps")
        nc.tensor.transpose(pmT_ps, pm, ident[0:G, 0:G])
        pmT = singles.tile([F2, G], F32)
        nc.vector.tensor_copy(pmT, pmT_ps)

        wl_sb = singles.tile([F2, F2], F32)
        nc.sync.dma_start(wl_sb, wl_in)
        wlt_ps = ps.tile([F2, F2], F32, tag="ps")
        nc.tensor.transpose(wlt_ps, wl_sb, ident[0:F2, 0:F2])
        wlt = singles.tile([F2, F2], F32)
        nc.vector.tensor_copy(wlt, wlt_ps)
        bl_sb = singles.tile([F2, 1], F32)
        nc.sync.dma_start(bl_sb, bl_in.rearrange("(a b) -> a b", b=1))
        y1_ps = ps.tile([F2, G], F32, tag="ps")
        nc.tensor.matmul(y1_ps, wlt, pmT, start=True, stop=True)
        y1 = singles.tile([F2, G], F32)
        nc.scalar.activation(y1, y1_ps, mybir.ActivationFunctionType.Identity, bias=bl_sb)
        wc_sb = singles.tile([F2, 1], F32)
        nc.sync.dma_start(wc_sb, wc_in.rearrange("a b -> b a"))
        bc_sb = singles.tile([1, 1], F32)
        nc.sync.dma_start(bc_sb, bc_in.rearrange("(a b) -> a b", b=1))
        y2_ps = ps.tile([1, G], F32, tag="ps")
        nc.tensor.matmul(y2_ps, wc_sb, y1, start=True, stop=True)
        y2 = singles.tile([1, G], F32)
        nc.scalar.activation(y2, y2_ps, mybir.ActivationFunctionType.Identity, bias=bc_sb)
        nc.sync.dma_start(out.rearrange("(a b) -> a b", a=1), y2)
    nc.finalize()
    return nc


# --------------------------------------------------------------------------
# Entry point
# --------------------------------------------------------------------------

def _note(rr, name):
    global LAST_EXEC_NS
    ns = rr.exec_time_ns
    if ns is not None:
        EXEC_NS.append((name, ns, rr.instructions_and_trace[1] if rr.instructions_and_trace else None))
        LAST_EXEC_NS += ns

def kernel(x, edge_index, batch, W1, a1_src, a1_dst, b1, g1, be1,
           W2, a2_src, a2_dst, b2, g2, be2, Wl, bl, Wc, bc):
    _register_ops()
    x = np.asarray(x, np.float32)
    edge_index = np.asarray(edge_index)
    batch = np.asarray(batch)
    ident = np.eye(128, dtype=np.float32)

    plan = _prep_plan(edge_index)
    ghot = _prep_pool(batch)

    # ---- P0: table build -------------------------------------------------
    xpad = np.zeros((NCORES, SLP, F1), np.float32)
    for c in range(NCORES):
        xpad[c, :SL] = x[c * SL:(c + 1) * SL]
    nc0 = _build_p0()
    in0 = [{"xsl": xpad[c], "W1": np.asarray(W1, np.float32),
            "a1s": np.asarray(a1_src, np.float32), "a1d": np.asarray(a1_dst, np.float32),
            "ident": ident} for c in range(NCORES)]
    _rr = _RUN(nc0, in0, core_ids=list(range(NCORES)), trace=TRACE)
    _note(_rr, "P0")
    r0 = _rr.results
    t1_full = np.concatenate([r0[c]["t1slice"] for c in range(NCORES)], axis=0)

    # ---- P1: layer 1 -----------------------------------------------------
    nc1 = _build_msg_layer(plan, 1)
    in1 = [{"tlo": t1_full[:LOROWS], "thi": t1_full[LOROWS:],
            "idxlo": plan["idx_lo"][c], "idxhi": plan["idx_hi"][c],
            "dstloc": plan["dstloc"][c], "down": r0[c]["d1own"],
            "bias": np.asarray(b1, np.float32), "gamma": np.asarray(g1, np.float32),
            "beta": np.asarray(be1, np.float32), "ident": ident,
            "W2": np.asarray(W2, np.float32), "a2s": np.asarray(a2_src, np.float32),
            "a2d": np.asarray(a2_dst, np.float32)} for c in range(NCORES)]
    _rr = _RUN(nc1, in1, core_ids=list(range(NCORES)), trace=TRACE)
    _note(_rr, "P1")
    r1 = _rr.results
    t2_full = np.concatenate([r1[c]["t2slice"] for c in range(NCORES)], axis=0)

    # ---- P2: layer 2 + partial pool -------------------------------------
    nc2 = _build_msg_layer(plan, 2)
    in2 = [{"tlo": t2_full[:LOROWS], "thi": t2_full[LOROWS:],
            "idxlo": plan["idx_lo"][c], "idxhi": plan["idx_hi"][c],
            "dstloc": plan["dstloc"][c], "down": r1[c]["d2own"],
            "bias": np.asarray(b2, np.float32), "gamma": np.asarray(g2, np.float32),
            "beta": np.asarray(be2, np.float32), "ident": ident,
            "ghot": ghot[c]} for c in range(NCORES)]
    _rr = _RUN(nc2, in2, core_ids=list(range(NCORES)), trace=TRACE)
    _note(_rr, "P2")
    r2 = _rr.results
    pall = np.stack([r2[c]["pooled"] for c in range(NCORES)], axis=0)
    pall = np.ascontiguousarray(pall.transpose(1, 0, 2).reshape(G, NCORES * (F2 + 1)))

    # ---- P3: combine + MLP ----------------------------------------------
    nc3 = _build_p3()
    in3 = [{"pall": pall, "Wl": np.asarray(Wl, np.float32),
            "bl": np.asarray(bl, np.float32), "Wc": np.asarray(Wc, np.float32),
            "bc": np.asarray(bc, np.float32), "ident": ident} for c in range(NCORES)]
    _rr = _RUN(nc3, in3, core_ids=list(range(NCORES)), trace=TRACE)
    _note(_rr, "P3")
    r3 = _rr.results
    return np.asarray(r3[0]["out"], np.float32)

